# revision 1
# baseline (speedup 1.0000x reference)
"""Multi-head GAT layer (4 heads, mean-aggregated) + residual + GraphNorm + gelu
on 8 Trainium2 NeuronCores (SPMD, one NEFF on all cores).

Strategy:
  - dst nodes partitioned contiguously across the 8 cores (12500 each); every
    edge is processed by the core owning its destination.
  - Every core runs one fused matmul sweep over ALL nodes producing per-node
    records [xl(256) | a_src(4) | pad] into a DRAM gather table (4 chunk
    tensors of 25088 rows so edge gathers can start before the whole table is
    written). A second tiny sweep over the core's OWN nodes produces a_dst and
    the residual.
  - Edge phase: per 128-dst tile, dma_gather (4 SWDGE queues, int16 chunk-local
    indices) pulls the per-edge src records; softmax over incoming edges is
    computed without the max shift (same result; alpha is bounded here); the
    segmented sum is a one-hot matmul into PSUM with the softmax denominator
    folded in as 4 extra columns.
  - GraphNorm: per-graph sums of h and h^2 via one-hot matmuls, AllReduced
    across cores, then per-node affine + gelu.

The edge structure (group sizes per tile/chunk) is made identical across cores
by padding each group to the max over cores, so one SPMD NEFF serves all 8.
"""
import numpy as np
import ml_dtypes

_BF16 = ml_dtypes.bfloat16

N, F, C, H, E, B = 100000, 128, 64, 4, 1200000, 8
NCORE = 8
NEG = 0.2
EPS = 1e-5
NOWN = N // NCORE             # 12500 dst nodes per core
TILES = (NOWN + 127) // 128   # 98 dst tiles per core (last partial: 84 rows)
LAST_ROWS = NOWN - (TILES - 1) * 128
NT = (N + 127) // 128         # 782 node tiles
NPAD = NT * 128               # 100096
NCH = 4
CHR = 25088                   # chunk rows; NCH*CHR >= NPAD
NTC = CHR // 128              # 196 node tiles per chunk
REC = 384                     # record elems: [xl 256 | asrc 4 | junk] (bf16: 768B)

_F32 = np.float32


def _cdiv(a, b):
    return (a + b - 1) // b


def _host_prep(x, edge_index, batch, W, att_src, att_dst, bias_gat, res_W,
               res_b, gn_weight, gn_bias, gn_mean_scale):
    """Compute static structure + per-core input tensors."""
    x = np.asarray(x, _F32)
    W = np.asarray(W, _F32)
    att_src = np.asarray(att_src, _F32)
    att_dst = np.asarray(att_dst, _F32)
    res_W = np.asarray(res_W, _F32)
    batch = np.asarray(batch).astype(np.int64)

    # fused node-sweep right matrix [F, 328] = [W.T | As.T | Ad.T | res_W.T]
    W3 = W.reshape(H, C, F)
    As = (att_src[:, :, None] * W3).sum(1)          # [H, F]
    Ad = (att_dst[:, :, None] * W3).sum(1)          # [H, F]
    Rcat = np.concatenate([W.T, As.T, Ad.T, res_W.T], axis=1).astype(_BF16)

    xT = np.zeros((F, NPAD), _BF16)
    xT[:, :N] = x.T.astype(_BF16)

    # ---- edges (+ self loops), assigned to cores by dst ----
    loop = np.arange(N, dtype=np.int64)
    src = np.concatenate([np.asarray(edge_index[0]), loop]).astype(np.int64)
    dst = np.concatenate([np.asarray(edge_index[1]), loop]).astype(np.int64)
    owner = dst // NOWN
    tl = (dst % NOWN) // 128
    dl = (dst % NOWN) % 128
    ch = src // CHR

    key = (owner * TILES + tl) * NCH + ch
    counts = np.bincount(key, minlength=NCORE * TILES * NCH).reshape(
        NCORE, TILES, NCH)
    K_tc = counts.max(axis=0).astype(np.int64)       # [TILES, NCH]
    nb_tc = _cdiv(K_tc, 128)                         # gather blocks per group
    n_sub = nb_tc.sum(axis=1)                        # [TILES]
    ic_tc = _cdiv(K_tc, 16)                          # idx cols per group

    B0 = np.zeros((TILES, NCH), np.int64)            # block offset within tile
    B0[:, 1:] = np.cumsum(nb_tc, axis=1)[:, :-1]
    jb = np.zeros(TILES + 1, np.int64)               # dstloc col offset per tile
    jb[1:] = np.cumsum(n_sub)
    NSUBTOT = int(jb[-1])
    O = np.zeros(TILES * NCH, np.int64)              # idx16 col offsets
    O_flat = np.cumsum(ic_tc.flatten())
    O[1:] = O_flat[:-1]
    O = O.reshape(TILES, NCH)
    IDXC = int(O_flat[-1])

    order = np.lexsort((ch, tl, owner))
    s_src, s_dl = src[order], dl[order]
    gstart = np.zeros(NCORE * TILES * NCH + 1, np.int64)
    gstart[1:] = np.cumsum(counts.flatten())

    in_maps = []
    for k in range(NCORE):
        idx16 = np.zeros((128, IDXC), np.int16)
        dstloc = np.full((128, NSUBTOT), -1.0, _F32)  # f32; cast at upload
        for t in range(TILES):
            for c in range(NCH):
                K = int(K_tc[t, c])
                if K == 0:
                    continue
                gi = (k * TILES + t) * NCH + c
                n = int(counts[k, t, c])
                a = int(gstart[gi])
                loc = (s_src[a:a + n] - c * CHR).astype(np.int16)
                padded = np.zeros(_cdiv(K, 16) * 16, np.int16)
                padded[:n] = loc
                blk = padded.reshape(-1, 16).T       # [16, icols]
                cols = blk.shape[1]
                idx16[:, O[t, c]:O[t, c] + cols] = np.tile(blk, (8, 1))
                dlv = np.full(int(nb_tc[t, c]) * 128, -1.0, _F32)
                dlv[:n] = s_dl[a:a + n].astype(_F32)
                j0 = int(jb[t] + B0[t, c])
                dstloc[:, j0:j0 + int(nb_tc[t, c])] = dlv.reshape(-1, 128).T

        # transposed one-hot M2_all[d, (j,e)] = (dstloc[e, j] == d), bf16
        m2 = (dstloc.T[:, :, None] == np.arange(128, dtype=_F32)[None, None, :])
        m2d = m2.transpose(2, 0, 1)          # [128 d, NSUBTOT, 128 e]
        m1d = m2.transpose(1, 0, 2)          # [128 e, NSUBTOT, 128 d]
        parts = []
        for t in range(TILES):
            j0, j1 = int(jb[t]), int(jb[t + 1])
            parts.append(m1d[:, j0:j1].reshape(128, -1))
            parts.append(m2d[:, j0:j1].reshape(128, -1))
        m12_all = np.ascontiguousarray(
            np.concatenate(parts, axis=1)).astype(_BF16)
        base = k * NOWN
        xTo = np.zeros((F, TILES * 128), _BF16)
        xTo[:, :NOWN] = x[base:base + NOWN].T.astype(_BF16)
        bslice = batch[base:base + NOWN]
        onehot_b = np.zeros((128, TILES * 8), _BF16)
        onehotT = np.zeros((8, TILES * 128), _F32)
        node_ids = np.arange(NOWN)
        pp = node_ids % 128
        tt = node_ids // 128
        onehot_b[pp, tt * 8 + bslice] = 1.0
        onehotT[bslice, node_ids] = 1.0

        in_maps.append({
            "xT": xT, "xTo": xTo, "Rcat": Rcat, "idx16": idx16,
            "dstloc": dstloc.astype(_BF16), "onehot_b": onehot_b,
            "onehotT": onehotT, "m12_all": m12_all,
        })

    bc_row = np.tile((np.asarray(bias_gat, _F32)
                      + np.asarray(res_b, _F32))[None, :], (128, 1))
    iota_row = np.tile(np.arange(128, dtype=_BF16)[None, :], (128, 1))
    iota_part = np.arange(128, dtype=_F32)[:, None].copy()
    ident = np.eye(128, dtype=_F32)
    alpha_t = np.full((128, 1), NEG, _F32)
    gms = np.asarray(gn_mean_scale, _F32)
    cnt = np.bincount(batch, minlength=B).astype(_F32)
    gn_pack = np.zeros((8, 4 * C + 2), _F32)
    gn_pack[:, 0:C] = np.asarray(gn_weight, _F32)[None, :]
    gn_pack[:, C:2 * C] = np.asarray(gn_bias, _F32)[None, :]
    gn_pack[:, 2 * C:3 * C] = gms[None, :]
    gn_pack[:, 3 * C:4 * C] = (gms * (2.0 - gms))[None, :]
    gn_pack[:, 4 * C] = 1.0 / cnt
    gn_pack[:, 4 * C + 1] = EPS
    for m in in_maps:
        m.update({"bc_row": bc_row, "iota_row": iota_row,
                  "iota_part": iota_part, "ident": ident,
                  "alpha_t": alpha_t, "gn_pack": gn_pack})

    cfg = {
        "K_tc": K_tc, "nb_tc": nb_tc, "n_sub": n_sub, "B0": B0, "jb": jb,
        "O": O, "ic_tc": ic_tc, "NSUBTOT": NSUBTOT, "IDXC": IDXC,
        "MAXSUB": int(n_sub.max()),
    }
    return cfg, in_maps


def _build_nc(cfg):
    import concourse.bacc as bacc
    import concourse.mybir as mybir
    import concourse.tile as tile

    AF = mybir.ActivationFunctionType
    OP = mybir.AluOpType
    f32 = mybir.dt.float32
    bf16 = mybir.dt.bfloat16
    i16 = mybir.dt.int16

    K_tc, nb_tc, n_sub = cfg["K_tc"], cfg["nb_tc"], cfg["n_sub"]
    B0, jb, O = cfg["B0"], cfg["jb"], cfg["O"]
    NSUBTOT, IDXC, MAXSUB = cfg["NSUBTOT"], cfg["IDXC"], cfg["MAXSUB"]

    nc = bacc.Bacc("TRN2", target_bir_lowering=False, num_swdge_queues=4,
                   dynamic_dma_scratch_size=40960)

    xT = nc.declare_dram_parameter("xT", [F, NPAD], bf16, isOutput=False)
    xTo = nc.declare_dram_parameter("xTo", [F, TILES * 128], bf16, isOutput=False)
    Rcat = nc.declare_dram_parameter("Rcat", [F, 328], bf16, isOutput=False)
    idx16 = nc.declare_dram_parameter("idx16", [128, IDXC], i16, isOutput=False)
    dstloc = nc.declare_dram_parameter("dstloc", [128, NSUBTOT], bf16, isOutput=False)
    onehot_b = nc.declare_dram_parameter("onehot_b", [128, TILES * 8], bf16, isOutput=False)
    onehotT = nc.declare_dram_parameter("onehotT", [8, TILES * 128], f32, isOutput=False)
    bc_row = nc.declare_dram_parameter("bc_row", [128, C], f32, isOutput=False)
    iota_row = nc.declare_dram_parameter("iota_row", [128, 128], bf16, isOutput=False)
    iota_part = nc.declare_dram_parameter("iota_part", [128, 1], f32, isOutput=False)
    ident = nc.declare_dram_parameter("ident", [128, 128], f32, isOutput=False)
    alpha_t = nc.declare_dram_parameter("alpha_t", [128, 1], f32, isOutput=False)
    gn_pack = nc.declare_dram_parameter("gn_pack", [8, 4 * C + 2], f32, isOutput=False)
    m12_all = nc.declare_dram_parameter("m12_all", [128, NSUBTOT * 256], bf16, isOutput=False)
    out = nc.declare_dram_parameter("out", [NOWN, C], f32, isOutput=True)

    tables = [nc.dram_tensor(f"table{c}", [CHR, REC], bf16) for c in range(NCH)]
    cc_in = nc.dram_tensor("cc_in", [8, 2 * C], f32)
    cc_out = nc.dram_tensor("cc_out", [8, 2 * C], f32)

    GRP = 7          # node tiles per phase-1 write group (196 = 28*7)
    NGRP = NTC // GRP
    qn = [0]

    def next_q():
        q = qn[0]
        qn[0] = (q + 1) % 4
        return q

    with tile.TileContext(nc) as tc:
        with (
            tc.tile_pool(name="const", bufs=1) as cp,
            tc.tile_pool(name="persist", bufs=1) as pers,
            tc.tile_pool(name="xload", bufs=2) as xp,
            tc.tile_pool(name="recw", bufs=2) as rp,
            tc.tile_pool(name="gat", bufs=5) as gp,
            tc.tile_pool(name="m1", bufs=3) as m1p,
            tc.tile_pool(name="m2", bufs=2) as m2p,
            tc.tile_pool(name="rhs", bufs=8) as rhp,
            tc.tile_pool(name="small", bufs=4) as smp,
            tc.tile_pool(name="idxp", bufs=6) as ixp,
        ):
            # ---- constants into SBUF ----
            rc_sb = cp.tile([F, 328], bf16)
            nc.sync.dma_start(rc_sb[:], Rcat[:])
            bc_sb = cp.tile([128, C], f32)
            nc.sync.dma_start(bc_sb[:], bc_row[:])
            ohb_sb = cp.tile([128, TILES * 8], bf16)
            nc.sync.dma_start(ohb_sb[:], onehot_b[:])
            al_sb = cp.tile([128, 1], f32)
            nc.sync.dma_start(al_sb[:], alpha_t[:])
            gn_sb = cp.tile([8, 4 * C + 2], f32)
            nc.sync.dma_start(gn_sb[:], gn_pack[:])

            adst_sb = pers.tile([128, TILES * 4], f32)
            resid_sb = pers.tile([128, TILES * C], bf16)
            h_sb = pers.tile([128, TILES * C], bf16)
            stats_sb = pers.tile([8, 2 * C], f32)
            nc.vector.memset(stats_sb[:], 0.0)

            with tc.tile_pool(name="psum1", bufs=3, space="PSUM") as ps1:
                # ---- phase 1b: owned-node sweep -> a_dst + residual ----
                for t in range(TILES):
                    xs = xp.tile([F, 128], bf16, tag="xo")
                    nc.sync.dma_start(xs[:], xTo[:, t * 128:(t + 1) * 128])
                    ps = ps1.tile([128, 68], f32, tag="ops")
                    nc.tensor.matmul(ps[:], lhsT=xs[:], rhs=rc_sb[:, 260:328],
                                     start=True, stop=True)
                    nc.vector.tensor_copy(adst_sb[:, t * 4:(t + 1) * 4],
                                          ps[:, 0:4])
                    nc.vector.tensor_tensor(
                        out=resid_sb[:, t * C:(t + 1) * C],
                        in0=ps[:, 4:68], in1=bc_sb[:], op=OP.add)

                # ---- phase 1a: record table build (all nodes) ----
                for c in range(NCH):
                    ntc_real = min(NTC, NT - c * NTC)
                    for g0 in range(0, ntc_real, GRP):
                        ng = min(GRP, ntc_real - g0)
                        t0 = c * NTC + g0
                        xs = xp.tile([F, GRP * 128], bf16, tag="x")
                        nc.scalar.dma_start(
                            xs[:, 0:ng * 128], xT[:, t0 * 128:(t0 + ng) * 128])
                        rec = rp.tile([128, GRP * REC], bf16, tag="rec")
                        for i in range(ng):
                            ps = ps1.tile([128, 328], f32, tag="nps")
                            nc.tensor.matmul(
                                ps[:], lhsT=xs[:, i * 128:(i + 1) * 128],
                                rhs=rc_sb[:], start=True, stop=True)
                            nc.scalar.copy(
                                rec[:, i * REC:i * REC + 328], ps[:, 0:328])
                        nc.scalar.dma_start(
                            tables[c][g0 * 128:(g0 + ng) * 128, :]
                            .rearrange("(i p) e -> p i e", p=128),
                            rec[:, 0:ng * REC].rearrange("p (i e) -> p i e", e=REC))

            # ---- phase 2: edge sweep over owned dst tiles ----
            with (
                tc.tile_pool(name="psum_adst", bufs=2, space="PSUM") as psa,
                tc.tile_pool(name="psum_agg", bufs=3, space="PSUM") as psg,
                tc.tile_pool(name="psum_dn", bufs=2, space="PSUM") as psn,
                tc.tile_pool(name="psum_stat", bufs=1, space="PSUM") as pss,
            ):
                for t in range(TILES):
                    ns = int(n_sub[t])
                    J = int(jb[t])
                    g = gp.tile([128, MAXSUB * REC], bf16, tag="g")
                    if t < 5:
                        # prime the pool slots: later tiles inherit old
                        # (finite) gather data in any region they don't write,
                        # but virgin SBUF may hold NaN bit patterns.
                        nc.vector.memset(g[:], 0.0)
                    o_t0 = int(O[t, 0])
                    oc_t = int(sum(_cdiv(int(K_tc[t, c]), 16) for c in range(NCH)))
                    ix = ixp.tile([128, oc_t], i16, tag="ix")
                    nc.sync.dma_start(ix[:], idx16[:, o_t0:o_t0 + oc_t])
                    for c in range(NCH):
                        K = int(K_tc[t, c])
                        if K == 0:
                            continue
                        nb = int(nb_tc[t, c])
                        b0 = int(B0[t, c])
                        oo = int(O[t, c]) - o_t0
                        oc = _cdiv(K, 16)
                        nc.gpsimd.dma_gather(
                            out_ap=g[:, b0 * REC:(b0 + nb) * REC]
                            .rearrange("p (j e) -> p j e", e=REC),
                            in_ap=tables[c][:],
                            idxs_ap=ix[:, oo:oo + oc],
                            num_idxs=K, num_idxs_reg=K,
                            elem_size=REC, queue_num=next_q())
                    # M1 | M2 one-hots uploaded from host, one DMA per tile
                    m12 = m1p.tile([128, 2 * MAXSUB * 128], bf16, tag="m1")
                    nc.sync.dma_start(m12[:, 0:2 * ns * 128],
                                      m12_all[:, J * 256:J * 256 + 2 * ns * 128])
                    m1 = m12[:, 0:ns * 128]
                    m2t = m12[:, ns * 128:2 * ns * 128]
                    adstb = smp.tile([128, 4], bf16, tag="adstb")
                    nc.vector.tensor_copy(adstb[:], adst_sb[:, t * 4:(t + 1) * 4])
                    ape = psa.tile([128, MAXSUB * 4], f32, tag="ape")
                    for j in range(ns):
                        nc.tensor.matmul(
                            ape[:, j * 4:(j + 1) * 4],
                            lhsT=m2t[:, j * 128:(j + 1) * 128],
                            rhs=adstb[:], start=True, stop=True)
                    # alpha = asrc[src] + adst[dst]; ex = exp(leakyrelu(alpha))
                    lr = smp.tile([128, MAXSUB * 4], f32, tag="lr")
                    nc.vector.tensor_tensor(
                        out=lr[:, 0:ns * 4].rearrange("p (j h) -> p j h", h=4),
                        in0=g[:].rearrange("p (j e) -> p j e", e=REC)[:, 0:ns, 256:260],
                        in1=ape[:, 0:ns * 4].rearrange("p (j h) -> p j h", h=4),
                        op=OP.add)
                    ex = smp.tile([128, MAXSUB * 4], bf16, tag="ex")
                    nc.scalar.activation(out=ex[:, 0:ns * 4], in_=lr[:, 0:ns * 4],
                                         func=AF.Prelu, alpha=al_sb[:, 0:1])
                    nc.scalar.activation(out=ex[:, 0:ns * 4], in_=ex[:, 0:ns * 4],
                                         func=AF.Exp)
                    # weighted segsum via one-hot matmuls; denominators are a
                    # second tiny matmul into cols 256:260 of the same bank
                    agg = psg.tile([128, 256], f32, tag="agg")
                    dnp = psn.tile([128, 4], f32, tag="dnp")
                    for j in range(ns):
                        rhs = rhp.tile([128, 256], bf16, tag="rhs")
                        nc.vector.tensor_tensor(
                            out=rhs[:],
                            in0=g[:, j * REC:j * REC + 256],
                            in1=ex[:, j * 4:(j + 1) * 4].to_broadcast([128, 4, C]),
                            op=OP.mult)
                        nc.tensor.matmul(
                            agg[:], lhsT=m1[:, j * 128:(j + 1) * 128],
                            rhs=rhs[:],
                            start=(j == 0), stop=(j == ns - 1))
                        nc.tensor.matmul(
                            dnp[:], lhsT=m1[:, j * 128:(j + 1) * 128],
                            rhs=ex[:, j * 4:(j + 1) * 4],
                            start=(j == 0), stop=(j == ns - 1))
                    # combine heads: h = 0.25*sum_h agg_h/denom_h + resid(+bias)
                    dn = smp.tile([128, 4], f32, tag="dn")
                    nc.vector.tensor_scalar(
                        out=dn[:], in0=dnp[:], scalar1=1e-6,
                        scalar2=None, op0=OP.add)
                    recip = smp.tile([128, 4], f32, tag="recip")
                    nc.vector.reciprocal(recip[:], dn[:])
                    hacc = smp.tile([128, C], f32, tag="hacc")
                    nc.vector.tensor_scalar(
                        out=hacc[:], in0=agg[:, 0:C], scalar1=recip[:, 0:1],
                        scalar2=None, op0=OP.mult)
                    for h in range(1, H):
                        nc.vector.scalar_tensor_tensor(
                            out=hacc[:], in0=agg[:, h * C:(h + 1) * C],
                            scalar=recip[:, h:h + 1], in1=hacc[:],
                            op0=OP.mult, op1=OP.add)
                    hsl = h_sb[:, t * C:(t + 1) * C]
                    nc.vector.scalar_tensor_tensor(
                        out=hsl, in0=hacc[:], scalar=1.0 / H,
                        in1=resid_sb[:, t * C:(t + 1) * C],
                        op0=OP.mult, op1=OP.add)
                    # graphnorm partial stats
                    sq = smp.tile([128, C], bf16, tag="sq")
                    nc.scalar.square(sq[:], hsl)
                    st = pss.tile([8, 2 * C], f32, tag="st")
                    nc.tensor.matmul(st[:, 0:C], lhsT=ohb_sb[:, t * 8:(t + 1) * 8],
                                     rhs=hsl, start=True, stop=True)
                    nc.tensor.matmul(st[:, C:2 * C], lhsT=ohb_sb[:, t * 8:(t + 1) * 8],
                                     rhs=sq[:], start=True, stop=True)
                    nc.vector.tensor_tensor(out=stats_sb[:], in0=stats_sb[:],
                                            in1=st[:], op=OP.add)

            # ---- phase 3: AllReduce stats, normalize, gelu, write out ----
            with tc.tile_pool(name="psum3", bufs=2, space="PSUM") as ps3:
                nc.gpsimd.dma_start(cc_in[:], stats_sb[:])
                nc.gpsimd.collective_compute(
                    "AllReduce", OP.add,
                    replica_groups=[list(range(NCORE))],
                    ins=[cc_in[:]], outs=[cc_out[:]])
                sall = smp.tile([8, 2 * C], f32, tag="sall")
                nc.sync.dma_start(sall[:], cc_out[:])
                gw = gn_sb[:, 0:C]
                gb = gn_sb[:, C:2 * C]
                gms = gn_sb[:, 2 * C:3 * C]
                gms2m = gn_sb[:, 3 * C:4 * C]
                invc = gn_sb[:, 4 * C:4 * C + 1]
                epsc = gn_sb[:, 4 * C + 1:4 * C + 2]
                mean = smp.tile([8, C], f32, tag="mean")
                nc.vector.tensor_scalar(out=mean[:], in0=sall[:, 0:C],
                                        scalar1=invc, scalar2=None, op0=OP.mult)
                eh2 = smp.tile([8, C], f32, tag="eh2")
                nc.vector.tensor_scalar(out=eh2[:], in0=sall[:, C:2 * C],
                                        scalar1=invc, scalar2=None, op0=OP.mult)
                msq = smp.tile([8, C], f32, tag="msq")
                nc.vector.tensor_tensor(out=msq[:], in0=mean[:], in1=mean[:],
                                        op=OP.mult)
                var = smp.tile([8, C], f32, tag="var")
                # var = eh2 - msq*gms2m
                nc.vector.tensor_tensor(out=msq[:], in0=msq[:], in1=gms2m,
                                        op=OP.mult)
                nc.vector.tensor_tensor(out=var[:], in0=eh2[:], in1=msq[:],
                                        op=OP.subtract)
                std = smp.tile([8, C], f32, tag="std")
                nc.scalar.activation(out=std[:], in_=var[:], func=AF.Sqrt,
                                     bias=epsc)
                ab = smp.tile([8, 2 * C], f32, tag="ab")
                nc.vector.reciprocal(std[:], std[:])
                # A = gw/std ; B = gb - A*mean*gms
                nc.vector.tensor_tensor(out=ab[:, 0:C], in0=gw, in1=std[:],
                                        op=OP.mult)
                tm = smp.tile([8, C], f32, tag="tm")
                nc.vector.tensor_tensor(out=tm[:], in0=ab[:, 0:C], in1=mean[:],
                                        op=OP.mult)
                nc.vector.tensor_tensor(out=tm[:], in0=tm[:], in1=gms,
                                        op=OP.mult)
                nc.vector.tensor_tensor(out=ab[:, C:2 * C], in0=gb, in1=tm[:],
                                        op=OP.subtract)

                for t in range(TILES):
                    oht = ixp.tile([8, 128], f32, tag="oht")
                    nc.sync.dma_start(oht[:], onehotT[:, t * 128:(t + 1) * 128])
                    abpe = ps3.tile([128, 2 * C], f32, tag="abpe")
                    nc.tensor.matmul(abpe[:], lhsT=oht[:], rhs=ab[:],
                                     start=True, stop=True)
                    nrm = smp.tile([128, C], f32, tag="nrm")
                    nc.vector.tensor_tensor(out=nrm[:],
                                            in0=h_sb[:, t * C:(t + 1) * C],
                                            in1=abpe[:, 0:C], op=OP.mult)
                    nc.vector.tensor_tensor(out=nrm[:], in0=nrm[:],
                                            in1=abpe[:, C:2 * C], op=OP.add)
                    ot = smp.tile([128, C], f32, tag="ot")
                    nc.scalar.activation(out=ot[:], in_=nrm[:],
                                         func=AF.Gelu_apprx_tanh)
                    rows = 128 if t < TILES - 1 else LAST_ROWS
                    nc.sync.dma_start(out[t * 128:t * 128 + rows, :],
                                      ot[:rows, :])

    nc.compile()
    return nc


def kernel(**inputs):
    from concourse.bass_utils import run_bass_kernel_spmd

    cfg, in_maps = _host_prep(**inputs)
    nc = _build_nc(cfg)
    res = run_bass_kernel_spmd(nc, in_maps, core_ids=list(range(NCORE)))
    return np.concatenate([res.results[k]["out"] for k in range(NCORE)], axis=0)



# revision 16
# speedup vs baseline: 2.0761x; 2.0761x over previous
"""Multi-head GAT layer (4 heads, mean-aggregated) + residual + GraphNorm + gelu
on 8 Trainium2 NeuronCores (SPMD, one NEFF on all cores).

v3 strategy:
  - dst nodes are dealt to (core, tile, slot) round-robin by in-degree so per
    tile edge counts are balanced across cores (less SPMD max-padding).
  - NO on-device gather (SWDGE descriptor generation is serial on the Pool
    engine at ~2-8ns/descriptor -> ~1ms floor for per-edge gathers).  Instead
    the host lays out the per-edge source rows x[src_e] directly in transposed
    block-aligned order (xedgeT[f, edge_slot]) and the kernel streams them as
    bulk contiguous DMA.  Edge slots are grouped as 98 dst tiles x ~13 blocks
    of 128 edges.
  - Per 128-edge block, PE recomputes xl = x@W.T (and a_src = x@As.T); the
    one-hot-distributed a_dst (ape) accumulates onto a_src in one PSUM tile
    per dst tile, so leakyrelu+exp run batched per tile on ACT.
  - One-hot matrices (m1: [e,d], m2t: [d,e]) are uploaded as fp8 (exact 0/1)
    and used directly as matmul weights against bf16 moving operands.
  - The per-edge coefficient multiply (rhs = xl * ex) runs on two-block PSUM
    tiles and is split between DVE and ACT.
  - Softmax denominators ride as 4 extra columns of the agg matmul rhs.
  - GraphNorm stats accumulate in a persistent PSUM bank across all tiles,
    then AllReduce across the 8 cores.
"""
import numpy as np
import ml_dtypes

_BF16 = ml_dtypes.bfloat16
_FP8 = ml_dtypes.float8_e4m3
_F32 = np.float32

N, F, C, H, E, B = 100000, 128, 64, 4, 1200000, 8
NCORE = 8
NEG = 0.2
EPS = 1e-5
NOWN = N // NCORE             # 12500 dst nodes per core
TILES = (NOWN + 127) // 128   # 98 dst tiles per core (last has 84 slots)
LAST_ROWS = NOWN - (TILES - 1) * 128
G = 7                         # dst tiles per DMA group (98 = 14*7)
DVE_MOD = 5                   # duos with (idx % DVE_MOD) < DVE_CUT multiply on DVE
DVE_CUT = 3


def _cdiv(a, b):
    return (a + b - 1) // b


def _assign_nodes(deg):
    """Deal nodes (by descending degree) round-robin over the 784 (core,tile)
    pairs, honoring per-tile capacity. Returns [N] arrays core, tile, slot."""
    P = NCORE * TILES
    caps = np.full(P, 128, np.int64)
    caps[TILES - 1::TILES] = LAST_ROWS
    order = np.argsort(-deg, kind="stable")
    fill = np.zeros(P, np.int64)
    core = np.empty(N, np.int64)
    tile = np.empty(N, np.int64)
    slot = np.empty(N, np.int64)
    p = 0
    for n in order:
        while fill[p] >= caps[p]:
            p = (p + 1) % P
        core[n] = p // TILES
        tile[n] = p % TILES
        slot[n] = fill[p]
        fill[p] += 1
        p = (p + 1) % P
    return core, tile, slot


def _host_prep(x, edge_index, batch, W, att_src, att_dst, bias_gat, res_W,
               res_b, gn_weight, gn_bias, gn_mean_scale):
    x = np.asarray(x, _F32)
    W = np.asarray(W, _F32)
    att_src = np.asarray(att_src, _F32)
    att_dst = np.asarray(att_dst, _F32)
    res_W = np.asarray(res_W, _F32)
    batch = np.asarray(batch).astype(np.int64)

    # fused right matrix [F, 328] = [W.T | As.T | Ad.T | res_W.T]
    W3 = W.reshape(H, C, F)
    As = (att_src[:, :, None] * W3).sum(1)
    Ad = (att_dst[:, :, None] * W3).sum(1)
    Rcat = np.concatenate([W.T, As.T, Ad.T, res_W.T], axis=1).astype(_BF16)
    xT_bf = x.T.astype(_BF16)                        # [F, N]

    # ---- edges (+ self loops) ----
    loop = np.arange(N, dtype=np.int64)
    src = np.concatenate([np.asarray(edge_index[0]), loop]).astype(np.int64)
    dst = np.concatenate([np.asarray(edge_index[1]), loop]).astype(np.int64)
    deg = np.bincount(dst, minlength=N)
    n_core, n_tile, n_slot = _assign_nodes(deg)

    owner = n_core[dst]
    tl = n_tile[dst]
    dl = n_slot[dst]

    key = owner * TILES + tl
    counts = np.bincount(key, minlength=NCORE * TILES).reshape(NCORE, TILES)
    K_t = counts.max(axis=0).astype(np.int64)        # [TILES]
    nb_t = _cdiv(K_t, 128)
    K_pad = nb_t * 128

    order = np.lexsort((tl, owner))
    s_src, s_dl = src[order], dl[order]
    gstart = np.zeros(NCORE * TILES + 1, np.int64)
    gstart[1:] = np.cumsum(counts.flatten())

    # ---- static block bookkeeping (same on all cores) ----
    groups = [list(range(g0, min(g0 + G, TILES))) for g0 in range(0, TILES, G)]
    TOTBLK = int(nb_t.sum())
    tile_blocks = [[] for _ in range(TILES)]   # (global block idx, group col)
    seg_meta = []                              # per tile: (t, blk0)
    blk = 0
    group_B0 = []
    for gtiles in groups:
        group_B0.append(blk)
        xoff = 0
        for t in gtiles:
            nb = int(nb_t[t])
            seg_meta.append((t, blk))
            for k in range(nb):
                tile_blocks[t].append((blk, xoff + k * 128))
                blk += 1
            xoff += nb * 128
    assert blk == TOTBLK
    gb_per_group = [int(nb_t[np.array(g)].sum()) for g in groups]
    MAXGB = max(gb_per_group)
    MAXNST = max(len(tb) for tb in tile_blocks)

    own_all = []
    for k in range(NCORE):
        own = np.where(n_core == k)[0]
        own = own[np.argsort((n_tile[own] * 128 + n_slot[own]), kind="stable")]
        own_all.append(own)

    # ---- per-core tensors ----
    in_maps = []
    for k in range(NCORE):
        xedgeT = np.zeros((F, TOTBLK * 128), _BF16)
        dlflat = np.full(TOTBLK * 128, -1, np.int64)
        for (t, blk0) in seg_meta:
            gi = k * TILES + t
            n = int(counts[k, t])
            if n == 0:
                continue
            a = int(gstart[gi])
            sl = slice(blk0 * 128, blk0 * 128 + n)
            xedgeT[:, sl] = xT_bf[:, s_src[a:a + n]]
            dlflat[sl] = s_dl[a:a + n]
        dlb = dlflat.reshape(TOTBLK, 128)
        m1arr = (dlb[:, :, None] == np.arange(128)[None, None, :])
        m1_all = np.ascontiguousarray(
            m1arr.transpose(1, 0, 2).reshape(128, TOTBLK * 128)).astype(_FP8)
        m2t_all = np.ascontiguousarray(
            m1arr.transpose(2, 0, 1).reshape(128, TOTBLK * 128)).astype(_FP8)

        own = own_all[k]
        rowpos = n_tile[own] * 128 + n_slot[own]
        xTo = np.zeros((F, TILES * 128), _BF16)
        xTo[:, rowpos] = xT_bf[:, own]
        bown = batch[own]
        onehot_b = np.zeros((128, TILES * 8), _BF16)
        onehot_b[rowpos % 128, (rowpos // 128) * 8 + bown] = 1.0
        onehotT = np.zeros((8, TILES * 128), _F32)
        onehotT[bown, rowpos] = 1.0

        in_maps.append({
            "xedgeT": xedgeT, "Rcat": Rcat,
            "m1_all": m1_all, "m2t_all": m2t_all,
            "xTo": xTo, "onehot_b": onehot_b, "onehotT": onehotT,
        })

    bc_row = np.tile((np.asarray(bias_gat, _F32)
                      + np.asarray(res_b, _F32))[None, :], (128, 1))
    alpha_t = np.full((128, 1), NEG, _F32)
    gms = np.asarray(gn_mean_scale, _F32)
    cnt = np.bincount(batch, minlength=B).astype(_F32)
    gn_pack = np.zeros((8, 4 * C + 2), _F32)
    gn_pack[:, 0:C] = np.asarray(gn_weight, _F32)[None, :]
    gn_pack[:, C:2 * C] = np.asarray(gn_bias, _F32)[None, :]
    gn_pack[:, 2 * C:3 * C] = gms[None, :]
    gn_pack[:, 3 * C:4 * C] = (gms * (2.0 - gms))[None, :]
    gn_pack[:, 4 * C] = 1.0 / cnt
    gn_pack[:, 4 * C + 1] = EPS
    for m in in_maps:
        m.update({"bc_row": bc_row, "alpha_t": alpha_t, "gn_pack": gn_pack})

    cfg = {
        "groups": groups, "group_B0": group_B0, "gb_per_group": gb_per_group,
        "tile_blocks": tile_blocks, "TOTBLK": TOTBLK,
        "MAXGB": MAXGB, "MAXNST": MAXNST, "own_all": own_all, "nb_t": nb_t,
    }
    return cfg, in_maps


def _build_nc(cfg, debug=False):
    import concourse.bacc as bacc
    import concourse.mybir as mybir
    import concourse.tile as tile

    AF = mybir.ActivationFunctionType
    OP = mybir.AluOpType
    f32 = mybir.dt.float32
    bf16 = mybir.dt.bfloat16
    fp8 = mybir.dt.float8e4

    groups = cfg["groups"]
    group_B0 = cfg["group_B0"]
    gb_per_group = cfg["gb_per_group"]
    tile_blocks = cfg["tile_blocks"]
    TOTBLK = cfg["TOTBLK"]
    MAXGB, MAXNST = cfg["MAXGB"], cfg["MAXNST"]

    nc = bacc.Bacc("TRN2", target_bir_lowering=False)

    xedgeT = nc.declare_dram_parameter("xedgeT", [F, TOTBLK * 128], bf16, isOutput=False)
    Rcat = nc.declare_dram_parameter("Rcat", [F, 328], bf16, isOutput=False)
    m1_all = nc.declare_dram_parameter("m1_all", [128, TOTBLK * 128], fp8, isOutput=False)
    m2t_all = nc.declare_dram_parameter("m2t_all", [128, TOTBLK * 128], fp8, isOutput=False)
    xTo = nc.declare_dram_parameter("xTo", [F, TILES * 128], bf16, isOutput=False)
    onehot_b = nc.declare_dram_parameter("onehot_b", [128, TILES * 8], bf16, isOutput=False)
    onehotT = nc.declare_dram_parameter("onehotT", [8, TILES * 128], f32, isOutput=False)
    bc_row = nc.declare_dram_parameter("bc_row", [128, C], f32, isOutput=False)
    alpha_t = nc.declare_dram_parameter("alpha_t", [128, 1], f32, isOutput=False)
    gn_pack = nc.declare_dram_parameter("gn_pack", [8, 4 * C + 2], f32, isOutput=False)
    out = nc.declare_dram_parameter("out", [NOWN, C], f32, isOutput=True)

    cc_in = nc.dram_tensor("cc_in", [8, 2 * C], f32)
    cc_out = nc.dram_tensor("cc_out", [8, 2 * C], f32)
    if debug:
        dbg_h = nc.declare_dram_parameter("dbg_h", [128, TILES * C], f32, isOutput=True)
        dbg_adst = nc.declare_dram_parameter("dbg_adst", [128, TILES * 4], f32, isOutput=True)
        dbg_resid = nc.declare_dram_parameter("dbg_resid", [128, TILES * C], f32, isOutput=True)
        dbg_stats = nc.declare_dram_parameter("dbg_stats", [8, 2 * C], f32, isOutput=True)
        dbg_ex = nc.declare_dram_parameter("dbg_ex", [128, MAXNST * 4], f32, isOutput=True)

    with tile.TileContext(nc) as tc:
        with (
            tc.tile_pool(name="const", bufs=1) as cp,
            tc.tile_pool(name="persist", bufs=1) as pers,
            tc.tile_pool(name="xload", bufs=2) as xp,
            tc.tile_pool(name="xe", bufs=2) as xep,
            tc.tile_pool(name="m1pool", bufs=2) as mp1,
            tc.tile_pool(name="m2pool", bufs=2) as mp2,
            tc.tile_pool(name="rhsp", bufs=2) as rhp,
            tc.tile_pool(name="small", bufs=4) as smp,
        ):
            # ---- constants ----
            rc_sb = cp.tile([F, 328], bf16)
            nc.sync.dma_start(rc_sb[:], Rcat[:])
            bc_sb = cp.tile([128, C], f32)
            nc.sync.dma_start(bc_sb[:], bc_row[:])
            ohb_sb = cp.tile([128, TILES * 8], bf16)
            nc.sync.dma_start(ohb_sb[:], onehot_b[:])
            al_sb = cp.tile([128, 1], f32)
            nc.sync.dma_start(al_sb[:], alpha_t[:])
            gn_sb = cp.tile([8, 4 * C + 2], f32)
            nc.sync.dma_start(gn_sb[:], gn_pack[:])

            adst_sb = pers.tile([128, TILES * 4], f32)
            resid_sb = pers.tile([128, TILES * C], bf16)
            h_sb = pers.tile([128, TILES * C], bf16)

            with tc.tile_pool(name="psum1", bufs=3, space="PSUM") as ps1:
                # ---- phase 1: owned-node sweep -> a_dst + residual ----
                for t in range(TILES):
                    xs = xp.tile([F, 128], bf16, tag="xo")
                    nc.sync.dma_start(xs[:], xTo[:, t * 128:(t + 1) * 128])
                    ps = ps1.tile([128, 68], f32, tag="ops")
                    nc.tensor.matmul(ps[:], lhsT=xs[:], rhs=rc_sb[:, 260:328],
                                     start=True, stop=True)
                    nc.vector.tensor_copy(adst_sb[:, t * 4:(t + 1) * 4],
                                          ps[:, 0:4])
                    nc.vector.tensor_tensor(
                        out=resid_sb[:, t * C:(t + 1) * C],
                        in0=ps[:, 4:68], in1=bc_sb[:], op=OP.add)

            # ---- phase 2: edge sweep ----
            with (
                tc.tile_pool(name="psum_xl", bufs=3, space="PSUM") as pxl,
                tc.tile_pool(name="psum_lr", bufs=2, space="PSUM") as plr,
                tc.tile_pool(name="psum_agg", bufs=2, space="PSUM") as pag,
                tc.tile_pool(name="psum_stat", bufs=1, space="PSUM") as pst,
            ):
                stats_ps = pst.tile([8, 2 * C], f32)
                duo_ctr = [0]
                for gi, gtiles in enumerate(groups):
                    B0 = group_B0[gi]
                    GB = gb_per_group[gi]
                    xe = xep.tile([F, MAXGB * 128], bf16, tag="xe")
                    nc.sync.dma_start(xe[:, 0:GB * 128],
                                      xedgeT[:, B0 * 128:(B0 + GB) * 128])
                    m1s = mp1.tile([128, MAXGB * 128], fp8, tag="m1")
                    nc.sync.dma_start(m1s[:, 0:GB * 128],
                                      m1_all[:, B0 * 128:(B0 + GB) * 128])
                    m2s = mp2.tile([128, MAXGB * 128], fp8, tag="m2")
                    nc.sync.dma_start(m2s[:, 0:GB * 128],
                                      m2t_all[:, B0 * 128:(B0 + GB) * 128])

                    for t in gtiles:
                        blocks = tile_blocks[t]
                        nst = len(blocks)
                        adstb = smp.tile([128, 4], bf16, tag="adstb")
                        nc.vector.tensor_copy(adstb[:],
                                              adst_sb[:, t * 4:(t + 1) * 4])
                        # a_src + distributed a_dst -> lr (one psum tile)
                        ps_lr = plr.tile([128, MAXNST * 4], f32, tag="lr")
                        for bj, (Bg, gcol) in enumerate(blocks):
                            xel = xe[:, gcol:gcol + 128]
                            nc.tensor.matmul(
                                ps_lr[:, bj * 4:(bj + 1) * 4], lhsT=xel,
                                rhs=rc_sb[:, 256:260], start=True, stop=False,
                                skip_group_check=True)
                            mb = (Bg - B0) * 128
                            nc.tensor.matmul(
                                ps_lr[:, bj * 4:(bj + 1) * 4],
                                lhsT=m2s[:, mb:mb + 128],
                                rhs=adstb[:], start=False, stop=True,
                                skip_group_check=True)
                        ex32 = smp.tile([128, MAXNST * 4], f32, tag="ex32")
                        nc.scalar.activation(out=ex32[:, 0:nst * 4],
                                             in_=ps_lr[:, 0:nst * 4],
                                             func=AF.Prelu, alpha=al_sb[:, 0:1])
                        nc.scalar.activation(out=ex32[:, 0:nst * 4],
                                             in_=ex32[:, 0:nst * 4], func=AF.Exp)
                        if debug and t == 0:
                            nc.sync.dma_start(dbg_ex[:, 0:nst * 4],
                                              ex32[:, 0:nst * 4])
                        rhs_t = rhp.tile([128, MAXNST * 260], bf16, tag="rhs")
                        nc.vector.tensor_copy(
                            rhs_t[:, 0:nst * 260]
                            .rearrange("p (j x) -> p j x", x=260)[:, :, 256:260],
                            ex32[:, 0:nst * 4]
                            .rearrange("p (j h) -> p j h", h=4))
                        agg = pag.tile([128, 260], f32, tag="agg")
                        for d0 in range(0, nst, 2):
                            nd = min(2, nst - d0)
                            ps_xl = pxl.tile([128, 512], f32, tag="xlps")
                            for u in range(nd):
                                bj = d0 + u
                                Bg, gcol = blocks[bj]
                                nc.tensor.matmul(
                                    ps_xl[:, u * 256:(u + 1) * 256],
                                    lhsT=xe[:, gcol:gcol + 128],
                                    rhs=rc_sb[:, 0:256],
                                    start=True, stop=True,
                                    skip_group_check=True)
                            if duo_ctr[0] % DVE_MOD < DVE_CUT:
                                nc.vector.tensor_tensor(
                                    out=rhs_t[:, d0 * 260:(d0 + nd) * 260]
                                    .rearrange("p (j x) -> p j x", x=260)
                                    [:, :, 0:256]
                                    .rearrange("p j (h c) -> p j h c", h=H),
                                    in0=ps_xl[:, 0:nd * 256]
                                    .rearrange("p (j h c) -> p j h c", h=H, c=C),
                                    in1=ex32[:, d0 * 4:(d0 + nd) * 4]
                                    .rearrange("p (j h) -> p j h", h=4)
                                    .to_broadcast([128, nd, H, C]),
                                    op=OP.mult)
                            else:
                                for u in range(nd):
                                    bj = d0 + u
                                    for h in range(H):
                                        nc.scalar.activation(
                                            out=rhs_t[:, bj * 260 + h * C:
                                                      bj * 260 + (h + 1) * C],
                                            in_=ps_xl[:, u * 256 + h * C:
                                                      u * 256 + (h + 1) * C],
                                            func=AF.Copy,
                                            scale=ex32[:, bj * 4 + h:
                                                       bj * 4 + h + 1])
                            duo_ctr[0] += 1
                            for u in range(nd):
                                bj = d0 + u
                                Bg, gcol = blocks[bj]
                                mb = (Bg - B0) * 128
                                nc.tensor.matmul(
                                    agg[:], lhsT=m1s[:, mb:mb + 128],
                                    rhs=rhs_t[:, bj * 260:(bj + 1) * 260],
                                    start=(bj == 0), stop=(bj == nst - 1))
                        # combine heads, add residual
                        dn = smp.tile([128, 4], f32, tag="dn")
                        nc.vector.tensor_scalar(
                            out=dn[:], in0=agg[:, 256:260], scalar1=1e-6,
                            scalar2=None, op0=OP.add)
                        recip = smp.tile([128, 4], f32, tag="recip")
                        nc.vector.reciprocal(recip[:], dn[:])
                        hacc = smp.tile([128, C], f32, tag="hacc")
                        nc.vector.tensor_scalar(
                            out=hacc[:], in0=agg[:, 0:C], scalar1=recip[:, 0:1],
                            scalar2=None, op0=OP.mult)
                        for h in range(1, H):
                            nc.vector.scalar_tensor_tensor(
                                out=hacc[:], in0=agg[:, h * C:(h + 1) * C],
                                scalar=recip[:, h:h + 1], in1=hacc[:],
                                op0=OP.mult, op1=OP.add)
                        hsl = h_sb[:, t * C:(t + 1) * C]
                        nc.vector.scalar_tensor_tensor(
                            out=hsl, in0=hacc[:], scalar=1.0 / H,
                            in1=resid_sb[:, t * C:(t + 1) * C],
                            op0=OP.mult, op1=OP.add)
                        # graphnorm partial stats (accumulate in psum)
                        sq = smp.tile([128, C], bf16, tag="sq")
                        nc.scalar.square(sq[:], hsl)
                        nc.tensor.matmul(stats_ps[:, 0:C],
                                         lhsT=ohb_sb[:, t * 8:(t + 1) * 8],
                                         rhs=hsl, start=(t == 0),
                                         stop=(t == TILES - 1),
                                         skip_group_check=True)
                        nc.tensor.matmul(stats_ps[:, C:2 * C],
                                         lhsT=ohb_sb[:, t * 8:(t + 1) * 8],
                                         rhs=sq[:], start=(t == 0),
                                         stop=(t == TILES - 1),
                                         skip_group_check=True)
                stats_sb = pers.tile([8, 2 * C], f32)
                nc.vector.tensor_copy(stats_sb[:], stats_ps[:])
            if debug:
                nc.sync.dma_start(dbg_adst[:], adst_sb[:])
                nc.sync.dma_start(dbg_stats[:], stats_sb[:])
                nc.gpsimd.dma_start(dbg_resid[:], resid_sb[:])
                nc.gpsimd.dma_start(dbg_h[:], h_sb[:])

            # ---- phase 3: AllReduce stats, normalize, gelu, write out ----
            with tc.tile_pool(name="psum3", bufs=2, space="PSUM") as ps3, \
                 tc.tile_pool(name="ohtp", bufs=2) as ohp:
                nc.gpsimd.dma_start(cc_in[:], stats_sb[:])
                nc.gpsimd.collective_compute(
                    "AllReduce", OP.add,
                    replica_groups=[list(range(NCORE))],
                    ins=[cc_in[:]], outs=[cc_out[:]])
                sall = smp.tile([8, 2 * C], f32, tag="sall")
                nc.sync.dma_start(sall[:], cc_out[:])
                gw = gn_sb[:, 0:C]
                gb = gn_sb[:, C:2 * C]
                gms = gn_sb[:, 2 * C:3 * C]
                gms2m = gn_sb[:, 3 * C:4 * C]
                invc = gn_sb[:, 4 * C:4 * C + 1]
                epsc = gn_sb[:, 4 * C + 1:4 * C + 2]
                mean = smp.tile([8, C], f32, tag="mean")
                nc.vector.tensor_scalar(out=mean[:], in0=sall[:, 0:C],
                                        scalar1=invc, scalar2=None, op0=OP.mult)
                eh2 = smp.tile([8, C], f32, tag="eh2")
                nc.vector.tensor_scalar(out=eh2[:], in0=sall[:, C:2 * C],
                                        scalar1=invc, scalar2=None, op0=OP.mult)
                msq = smp.tile([8, C], f32, tag="msq")
                nc.vector.tensor_tensor(out=msq[:], in0=mean[:], in1=mean[:],
                                        op=OP.mult)
                var = smp.tile([8, C], f32, tag="var")
                nc.vector.tensor_tensor(out=msq[:], in0=msq[:], in1=gms2m,
                                        op=OP.mult)
                nc.vector.tensor_tensor(out=var[:], in0=eh2[:], in1=msq[:],
                                        op=OP.subtract)
                std = smp.tile([8, C], f32, tag="std")
                nc.scalar.activation(out=std[:], in_=var[:], func=AF.Sqrt,
                                     bias=epsc)
                ab = smp.tile([8, 2 * C], f32, tag="ab")
                nc.vector.reciprocal(std[:], std[:])
                nc.vector.tensor_tensor(out=ab[:, 0:C], in0=gw, in1=std[:],
                                        op=OP.mult)
                tm = smp.tile([8, C], f32, tag="tm")
                nc.vector.tensor_tensor(out=tm[:], in0=ab[:, 0:C], in1=mean[:],
                                        op=OP.mult)
                nc.vector.tensor_tensor(out=tm[:], in0=tm[:], in1=gms,
                                        op=OP.mult)
                nc.vector.tensor_tensor(out=ab[:, C:2 * C], in0=gb, in1=tm[:],
                                        op=OP.subtract)

                for t in range(TILES):
                    oht = ohp.tile([8, 128], f32, tag="oht")
                    nc.sync.dma_start(oht[:], onehotT[:, t * 128:(t + 1) * 128])
                    abpe = ps3.tile([128, 2 * C], f32, tag="abpe")
                    nc.tensor.matmul(abpe[:], lhsT=oht[:], rhs=ab[:],
                                     start=True, stop=True)
                    nrm = smp.tile([128, C], f32, tag="nrm")
                    nc.vector.tensor_tensor(out=nrm[:],
                                            in0=h_sb[:, t * C:(t + 1) * C],
                                            in1=abpe[:, 0:C], op=OP.mult)
                    nc.vector.tensor_tensor(out=nrm[:], in0=nrm[:],
                                            in1=abpe[:, C:2 * C], op=OP.add)
                    ot = smp.tile([128, C], f32, tag="ot")
                    nc.scalar.activation(out=ot[:], in_=nrm[:],
                                         func=AF.Gelu_apprx_tanh)
                    rows = 128 if t < TILES - 1 else LAST_ROWS
                    nc.sync.dma_start(out[t * 128:t * 128 + rows, :],
                                      ot[:rows, :])

    nc.compile()
    return nc


def kernel(**inputs):
    from concourse.bass_utils import run_bass_kernel_spmd

    cfg, in_maps = _host_prep(**inputs)
    nc = _build_nc(cfg)
    res = run_bass_kernel_spmd(nc, in_maps, core_ids=list(range(NCORE)))
    full = np.empty((N, C), _F32)
    for k in range(NCORE):
        full[cfg["own_all"][k]] = res.results[k]["out"]
    return full


# revision 28
# speedup vs baseline: 2.9338x; 1.4132x over previous
"""Multi-head GAT layer (4 heads, mean-aggregated) + residual + GraphNorm + gelu
on 8 Trainium2 NeuronCores (SPMD, one NEFF on all cores).

v3 strategy:
  - dst nodes are dealt to (core, tile, slot) round-robin by in-degree so per
    tile edge counts are balanced across cores (less SPMD max-padding).
  - NO on-device gather (SWDGE descriptor generation is serial on the Pool
    engine at ~2-8ns/descriptor -> ~1ms floor for per-edge gathers).  Instead
    the host lays out the per-edge source rows x[src_e] directly in transposed
    block-aligned order (xedgeT[f, edge_slot]) and the kernel streams them as
    bulk contiguous DMA.  Edge slots are grouped as 98 dst tiles x ~13 blocks
    of 128 edges.
  - Per 128-edge block, PE recomputes xl = x@W.T (and a_src = x@As.T); the
    one-hot-distributed a_dst (ape) accumulates onto a_src in one PSUM tile
    per dst tile, so leakyrelu+exp run batched per tile on ACT.
  - One-hot matrices (m1: [e,d], m2t: [d,e]) are uploaded as fp8 (exact 0/1)
    and used directly as matmul weights against bf16 moving operands.
  - The per-edge coefficient multiply (rhs = xl * ex) runs on two-block PSUM
    tiles and is split between DVE and ACT.
  - Softmax denominators ride as 4 extra columns of the agg matmul rhs.
  - GraphNorm stats accumulate in a persistent PSUM bank across all tiles,
    then AllReduce across the 8 cores.
"""
import numpy as np
import ml_dtypes

_BF16 = ml_dtypes.bfloat16
_FP8 = ml_dtypes.float8_e4m3
_F32 = np.float32

N, F, C, H, E, B = 100000, 128, 64, 4, 1200000, 8
NCORE = 8
NEG = 0.2
EPS = 1e-5
NOWN = N // NCORE             # 12500 dst nodes per core
TILES = (NOWN + 127) // 128   # 98 dst tiles per core (last has 84 slots)
LAST_ROWS = NOWN - (TILES - 1) * 128
G = 7                         # dst tiles per DMA group (98 = 14*7)
DVE_MOD = 8                   # duos with (idx % DVE_MOD) < DVE_CUT multiply on DVE
DVE_CUT = 7


def _cdiv(a, b):
    return (a + b - 1) // b


def _assign_nodes(deg):
    """Deal nodes (by descending degree) round-robin over the 784 (core,tile)
    pairs, honoring per-tile capacity. Returns [N] arrays core, tile, slot."""
    P = NCORE * TILES
    caps = np.full(P, 128, np.int64)
    caps[TILES - 1::TILES] = LAST_ROWS
    order = np.argsort(-deg, kind="stable")
    fill = np.zeros(P, np.int64)
    core = np.empty(N, np.int64)
    tile = np.empty(N, np.int64)
    slot = np.empty(N, np.int64)
    p = 0
    for n in order:
        while fill[p] >= caps[p]:
            p = (p + 1) % P
        core[n] = p // TILES
        tile[n] = p % TILES
        slot[n] = fill[p]
        fill[p] += 1
        p = (p + 1) % P
    return core, tile, slot


def _host_prep(x, edge_index, batch, W, att_src, att_dst, bias_gat, res_W,
               res_b, gn_weight, gn_bias, gn_mean_scale):
    x = np.asarray(x, _F32)
    W = np.asarray(W, _F32)
    att_src = np.asarray(att_src, _F32)
    att_dst = np.asarray(att_dst, _F32)
    res_W = np.asarray(res_W, _F32)
    batch = np.asarray(batch).astype(np.int64)

    # fused right matrix [F, 328] = [W.T | As.T | Ad.T | res_W.T]
    W3 = W.reshape(H, C, F)
    As = (att_src[:, :, None] * W3).sum(1)
    Ad = (att_dst[:, :, None] * W3).sum(1)
    Rcat = np.concatenate([W.T, As.T, Ad.T, res_W.T], axis=1).astype(_BF16)
    xT_bf = x.T.astype(_BF16)                        # [F, N]

    # ---- edges (+ self loops) ----
    loop = np.arange(N, dtype=np.int64)
    src = np.concatenate([np.asarray(edge_index[0]), loop]).astype(np.int64)
    dst = np.concatenate([np.asarray(edge_index[1]), loop]).astype(np.int64)
    deg = np.bincount(dst, minlength=N)
    n_core, n_tile, n_slot = _assign_nodes(deg)

    owner = n_core[dst]
    tl = n_tile[dst]
    dl = n_slot[dst]

    key = owner * TILES + tl
    counts = np.bincount(key, minlength=NCORE * TILES).reshape(NCORE, TILES)
    K_t = counts.max(axis=0).astype(np.int64)        # [TILES]
    nb_t = _cdiv(K_t, 128)
    K_pad = nb_t * 128

    order = np.lexsort((tl, owner))
    s_src, s_dl = src[order], dl[order]
    gstart = np.zeros(NCORE * TILES + 1, np.int64)
    gstart[1:] = np.cumsum(counts.flatten())

    # ---- static block bookkeeping (same on all cores) ----
    groups = [list(range(g0, min(g0 + G, TILES))) for g0 in range(0, TILES, G)]
    TOTBLK = int(nb_t.sum())
    tile_blocks = [[] for _ in range(TILES)]   # (global block idx, group col)
    seg_meta = []                              # per tile: (t, blk0)
    blk = 0
    group_B0 = []
    for gtiles in groups:
        group_B0.append(blk)
        xoff = 0
        for t in gtiles:
            nb = int(nb_t[t])
            seg_meta.append((t, blk))
            for k in range(nb):
                tile_blocks[t].append((blk, xoff + k * 128))
                blk += 1
            xoff += nb * 128
    assert blk == TOTBLK
    gb_per_group = [int(nb_t[np.array(g)].sum()) for g in groups]
    MAXGB = max(gb_per_group)
    MAXNST = max(len(tb) for tb in tile_blocks)

    own_all = []
    for k in range(NCORE):
        own = np.where(n_core == k)[0]
        own = own[np.argsort((n_tile[own] * 128 + n_slot[own]), kind="stable")]
        own_all.append(own)

    # ---- per-core tensors ----
    in_maps = []
    for k in range(NCORE):
        xedgeT = np.zeros((F, TOTBLK * 128), _BF16)
        dlflat = np.full(TOTBLK * 128, -1, np.int64)
        for (t, blk0) in seg_meta:
            gi = k * TILES + t
            n = int(counts[k, t])
            if n == 0:
                continue
            a = int(gstart[gi])
            sl = slice(blk0 * 128, blk0 * 128 + n)
            xedgeT[:, sl] = xT_bf[:, s_src[a:a + n]]
            dlflat[sl] = s_dl[a:a + n]
        dlb = dlflat.reshape(TOTBLK, 128)
        m1arr = (dlb[:, :, None] == np.arange(128)[None, None, :])
        m1_all = np.ascontiguousarray(
            m1arr.transpose(1, 0, 2).reshape(128, TOTBLK * 128)).astype(_FP8)
        m2t_all = np.ascontiguousarray(
            m1arr.transpose(2, 0, 1).reshape(128, TOTBLK * 128)).astype(_FP8)

        own = own_all[k]
        rowpos = n_tile[own] * 128 + n_slot[own]
        xTo = np.zeros((F, TILES * 128), _BF16)
        xTo[:, rowpos] = xT_bf[:, own]
        bown = batch[own]
        onehot_b = np.zeros((128, TILES * 8), _BF16)
        onehot_b[rowpos % 128, (rowpos // 128) * 8 + bown] = 1.0
        onehotT = np.zeros((8, TILES * 128), _F32)
        onehotT[bown, rowpos] = 1.0

        in_maps.append({
            "xedgeT": xedgeT, "Rcat": Rcat,
            "m1_all": m1_all, "m2t_all": m2t_all,
            "xTo": xTo, "onehot_b": onehot_b, "onehotT": onehotT,
        })

    bc_row = np.tile((np.asarray(bias_gat, _F32)
                      + np.asarray(res_b, _F32))[None, :], (128, 1))
    alpha_t = np.full((128, 1), NEG, _F32)
    gms = np.asarray(gn_mean_scale, _F32)
    cnt = np.bincount(batch, minlength=B).astype(_F32)
    gn_pack = np.zeros((8, 4 * C + 2), _F32)
    gn_pack[:, 0:C] = np.asarray(gn_weight, _F32)[None, :]
    gn_pack[:, C:2 * C] = np.asarray(gn_bias, _F32)[None, :]
    gn_pack[:, 2 * C:3 * C] = gms[None, :]
    gn_pack[:, 3 * C:4 * C] = (gms * (2.0 - gms))[None, :]
    gn_pack[:, 4 * C] = 1.0 / cnt
    gn_pack[:, 4 * C + 1] = EPS
    for m in in_maps:
        m.update({"bc_row": bc_row, "alpha_t": alpha_t, "gn_pack": gn_pack})

    cfg = {
        "groups": groups, "group_B0": group_B0, "gb_per_group": gb_per_group,
        "tile_blocks": tile_blocks, "TOTBLK": TOTBLK,
        "MAXGB": MAXGB, "MAXNST": MAXNST, "own_all": own_all, "nb_t": nb_t,
    }
    return cfg, in_maps


def _build_nc(cfg, debug=False):
    import concourse.bacc as bacc
    import concourse.mybir as mybir
    import concourse.tile as tile

    AF = mybir.ActivationFunctionType
    OP = mybir.AluOpType
    f32 = mybir.dt.float32
    bf16 = mybir.dt.bfloat16
    fp8 = mybir.dt.float8e4

    groups = cfg["groups"]
    group_B0 = cfg["group_B0"]
    gb_per_group = cfg["gb_per_group"]
    tile_blocks = cfg["tile_blocks"]
    TOTBLK = cfg["TOTBLK"]
    MAXGB, MAXNST = cfg["MAXGB"], cfg["MAXNST"]

    nc = bacc.Bacc("TRN2", target_bir_lowering=False)

    xedgeT = nc.declare_dram_parameter("xedgeT", [F, TOTBLK * 128], bf16, isOutput=False)
    Rcat = nc.declare_dram_parameter("Rcat", [F, 328], bf16, isOutput=False)
    m1_all = nc.declare_dram_parameter("m1_all", [128, TOTBLK * 128], fp8, isOutput=False)
    m2t_all = nc.declare_dram_parameter("m2t_all", [128, TOTBLK * 128], fp8, isOutput=False)
    xTo = nc.declare_dram_parameter("xTo", [F, TILES * 128], bf16, isOutput=False)
    onehot_b = nc.declare_dram_parameter("onehot_b", [128, TILES * 8], bf16, isOutput=False)
    onehotT = nc.declare_dram_parameter("onehotT", [8, TILES * 128], f32, isOutput=False)
    bc_row = nc.declare_dram_parameter("bc_row", [128, C], f32, isOutput=False)
    alpha_t = nc.declare_dram_parameter("alpha_t", [128, 1], f32, isOutput=False)
    gn_pack = nc.declare_dram_parameter("gn_pack", [8, 4 * C + 2], f32, isOutput=False)
    out = nc.declare_dram_parameter("out", [NOWN, C], f32, isOutput=True)

    cc_in = nc.dram_tensor("cc_in", [8, 2 * C], f32)
    cc_out = nc.dram_tensor("cc_out", [8, 2 * C], f32)
    if debug:
        dbg_h = nc.declare_dram_parameter("dbg_h", [128, TILES * C], f32, isOutput=True)
        dbg_adst = nc.declare_dram_parameter("dbg_adst", [128, TILES * 4], f32, isOutput=True)
        dbg_resid = nc.declare_dram_parameter("dbg_resid", [128, TILES * C], f32, isOutput=True)
        dbg_stats = nc.declare_dram_parameter("dbg_stats", [8, 2 * C], f32, isOutput=True)
        dbg_ex = nc.declare_dram_parameter("dbg_ex", [128, MAXNST * 4], f32, isOutput=True)

    with tile.TileContext(nc) as tc:
        with (
            tc.tile_pool(name="const", bufs=1) as cp,
            tc.tile_pool(name="persist", bufs=1) as pers,
            tc.tile_pool(name="xload", bufs=2) as xp,
            tc.tile_pool(name="xe", bufs=2) as xep,
            tc.tile_pool(name="m1pool", bufs=2) as mp1,
            tc.tile_pool(name="m2pool", bufs=2) as mp2,
            tc.tile_pool(name="rhsp", bufs=2) as rhp,
            tc.tile_pool(name="small", bufs=4) as smp,
        ):
            # ---- constants ----
            rc_sb = cp.tile([F, 328], bf16)
            nc.sync.dma_start(rc_sb[:], Rcat[:])
            bc_sb = cp.tile([128, C], f32)
            nc.sync.dma_start(bc_sb[:], bc_row[:])
            ohb_sb = cp.tile([128, TILES * 8], bf16)
            nc.sync.dma_start(ohb_sb[:], onehot_b[:])
            al_sb = cp.tile([128, 1], f32)
            nc.sync.dma_start(al_sb[:], alpha_t[:])
            gn_sb = cp.tile([8, 4 * C + 2], f32)
            nc.sync.dma_start(gn_sb[:], gn_pack[:])


            adst_sb = pers.tile([128, TILES * 4], f32)
            resid_sb = pers.tile([128, TILES * C], bf16)
            h_sb = pers.tile([128, TILES * C], bf16)

            with tc.tile_pool(name="psum1", bufs=3, space="PSUM") as ps1:
                # ---- phase 1: owned-node sweep -> a_dst + residual ----
                for g0 in range(0, TILES, G):
                    ng = min(G, TILES - g0)
                    xs = xp.tile([F, G * 128], bf16, tag="xo")
                    nc.sync.dma_start(xs[:, 0:ng * 128],
                                      xTo[:, g0 * 128:(g0 + ng) * 128])
                    for i in range(ng):
                        t = g0 + i
                        ps = ps1.tile([128, 68], f32, tag="ops")
                        nc.tensor.matmul(ps[:], lhsT=xs[:, i * 128:(i + 1) * 128],
                                         rhs=rc_sb[:, 260:328],
                                         start=True, stop=True)
                        nc.vector.tensor_copy(adst_sb[:, t * 4:(t + 1) * 4],
                                              ps[:, 0:4])
                        nc.vector.tensor_tensor(
                            out=resid_sb[:, t * C:(t + 1) * C],
                            in0=ps[:, 4:68], in1=bc_sb[:], op=OP.add)

            # ---- phase 2: edge sweep ----
            with (
                tc.tile_pool(name="psum_xl", bufs=3, space="PSUM") as pxl,
                tc.tile_pool(name="psum_lr", bufs=2, space="PSUM") as plr,
                tc.tile_pool(name="psum_agg", bufs=2, space="PSUM") as pag,
                tc.tile_pool(name="psum_stat", bufs=1, space="PSUM") as pst,
            ):
                stats_ps = pst.tile([8, 2 * C], f32)
                duo_ctr = [0]
                for gi, gtiles in enumerate(groups):
                    B0 = group_B0[gi]
                    GB = gb_per_group[gi]
                    xe = xep.tile([F, MAXGB * 128], bf16, tag="xe")
                    nc.sync.dma_start(xe[:, 0:GB * 128],
                                      xedgeT[:, B0 * 128:(B0 + GB) * 128])
                    m1s = mp1.tile([128, MAXGB * 128], fp8, tag="m1")
                    nc.sync.dma_start(m1s[:, 0:GB * 128],
                                      m1_all[:, B0 * 128:(B0 + GB) * 128])
                    m2s = mp2.tile([128, MAXGB * 128], fp8, tag="m2")
                    nc.sync.dma_start(m2s[:, 0:GB * 128],
                                      m2t_all[:, B0 * 128:(B0 + GB) * 128])

                    for t in gtiles:
                        blocks = tile_blocks[t]
                        nst = len(blocks)
                        adstb = smp.tile([128, 4], bf16, tag="adstb")
                        nc.vector.tensor_copy(adstb[:],
                                              adst_sb[:, t * 4:(t + 1) * 4])
                        # a_src + distributed a_dst -> lr (one psum tile)
                        ps_lr = plr.tile([128, MAXNST * 4], f32, tag="lr")
                        for bj, (Bg, gcol) in enumerate(blocks):
                            xel = xe[:, gcol:gcol + 128]
                            nc.tensor.matmul(
                                ps_lr[:, bj * 4:(bj + 1) * 4], lhsT=xel,
                                rhs=rc_sb[:, 256:260], start=True, stop=False,
                                skip_group_check=True)
                            mb = (Bg - B0) * 128
                            nc.tensor.matmul(
                                ps_lr[:, bj * 4:(bj + 1) * 4],
                                lhsT=m2s[:, mb:mb + 128],
                                rhs=adstb[:], start=False, stop=True,
                                skip_group_check=True)
                        ex32 = smp.tile([128, MAXNST * 4], f32, tag="ex32")
                        nc.scalar.activation(out=ex32[:, 0:nst * 4],
                                             in_=ps_lr[:, 0:nst * 4],
                                             func=AF.Prelu, alpha=al_sb[:, 0:1])
                        nc.scalar.activation(out=ex32[:, 0:nst * 4],
                                             in_=ex32[:, 0:nst * 4], func=AF.Exp)
                        if debug and t == 0:
                            nc.sync.dma_start(dbg_ex[:, 0:nst * 4],
                                              ex32[:, 0:nst * 4])
                        rhs_t = rhp.tile([128, MAXNST * 260], bf16, tag="rhs")
                        nc.vector.tensor_copy(
                            rhs_t[:, 0:nst * 260]
                            .rearrange("p (j x) -> p j x", x=260)[:, :, 256:260],
                            ex32[:, 0:nst * 4]
                            .rearrange("p (j h) -> p j h", h=4))
                        agg = pag.tile([128, 260], f32, tag="agg")
                        for d0 in range(0, nst, 2):
                            nd = min(2, nst - d0)
                            ps_xl = pxl.tile([128, 512], f32, tag="xlps")
                            for u in range(nd):
                                bj = d0 + u
                                Bg, gcol = blocks[bj]
                                nc.tensor.matmul(
                                    ps_xl[:, u * 256:(u + 1) * 256],
                                    lhsT=xe[:, gcol:gcol + 128],
                                    rhs=rc_sb[:, 0:256],
                                    start=True, stop=True,
                                    skip_group_check=True)
                            if duo_ctr[0] % DVE_MOD < DVE_CUT:
                                nc.vector.tensor_tensor(
                                    out=rhs_t[:, d0 * 260:(d0 + nd) * 260]
                                    .rearrange("p (j x) -> p j x", x=260)
                                    [:, :, 0:256]
                                    .rearrange("p j (h c) -> p j h c", h=H),
                                    in0=ps_xl[:, 0:nd * 256]
                                    .rearrange("p (j h c) -> p j h c", h=H, c=C),
                                    in1=ex32[:, d0 * 4:(d0 + nd) * 4]
                                    .rearrange("p (j h) -> p j h", h=4)
                                    .to_broadcast([128, nd, H, C]),
                                    op=OP.mult)
                            else:
                                for u in range(nd):
                                    bj = d0 + u
                                    for h in range(H):
                                        nc.scalar.activation(
                                            out=rhs_t[:, bj * 260 + h * C:
                                                      bj * 260 + (h + 1) * C],
                                            in_=ps_xl[:, u * 256 + h * C:
                                                      u * 256 + (h + 1) * C],
                                            func=AF.Copy,
                                            scale=ex32[:, bj * 4 + h:
                                                       bj * 4 + h + 1])
                            duo_ctr[0] += 1
                            for u in range(nd):
                                bj = d0 + u
                                Bg, gcol = blocks[bj]
                                mb = (Bg - B0) * 128
                                nc.tensor.matmul(
                                    agg[:], lhsT=m1s[:, mb:mb + 128],
                                    rhs=rhs_t[:, bj * 260:(bj + 1) * 260],
                                    start=(bj == 0), stop=(bj == nst - 1))
                        # combine heads, add residual
                        dn = smp.tile([128, 4], f32, tag="dn")
                        nc.vector.tensor_scalar(
                            out=dn[:], in0=agg[:, 256:260], scalar1=1e-6,
                            scalar2=None, op0=OP.add)
                        recip = smp.tile([128, 4], f32, tag="recip")
                        nc.vector.reciprocal(recip[:], dn[:])
                        hacc = smp.tile([128, C], f32, tag="hacc")
                        nc.vector.tensor_scalar(
                            out=hacc[:], in0=agg[:, 0:C], scalar1=recip[:, 0:1],
                            scalar2=None, op0=OP.mult)
                        for h in range(1, H):
                            nc.vector.scalar_tensor_tensor(
                                out=hacc[:], in0=agg[:, h * C:(h + 1) * C],
                                scalar=recip[:, h:h + 1], in1=hacc[:],
                                op0=OP.mult, op1=OP.add)
                        hsl = h_sb[:, t * C:(t + 1) * C]
                        nc.vector.scalar_tensor_tensor(
                            out=hsl, in0=hacc[:], scalar=1.0 / H,
                            in1=resid_sb[:, t * C:(t + 1) * C],
                            op0=OP.mult, op1=OP.add)
                        # graphnorm partial stats (accumulate in psum)
                        sq = smp.tile([128, C], bf16, tag="sq")
                        nc.scalar.square(sq[:], hsl)
                        nc.tensor.matmul(stats_ps[:, 0:C],
                                         lhsT=ohb_sb[:, t * 8:(t + 1) * 8],
                                         rhs=hsl, start=(t == 0),
                                         stop=(t == TILES - 1),
                                         skip_group_check=True)
                        nc.tensor.matmul(stats_ps[:, C:2 * C],
                                         lhsT=ohb_sb[:, t * 8:(t + 1) * 8],
                                         rhs=sq[:], start=(t == 0),
                                         stop=(t == TILES - 1),
                                         skip_group_check=True)
                stats_sb = pers.tile([8, 2 * C], f32)
                nc.vector.tensor_copy(stats_sb[:], stats_ps[:])
            if debug:
                nc.sync.dma_start(dbg_adst[:], adst_sb[:])
                nc.sync.dma_start(dbg_stats[:], stats_sb[:])
                nc.gpsimd.dma_start(dbg_resid[:], resid_sb[:])
                nc.gpsimd.dma_start(dbg_h[:], h_sb[:])

            # ---- phase 3: AllReduce stats, normalize, gelu, write out ----
            with tc.tile_pool(name="psum3", bufs=2, space="PSUM") as ps3, \
                 tc.tile_pool(name="ohtp", bufs=2) as ohp:
                nc.gpsimd.dma_start(cc_in[:], stats_sb[:])
                nc.gpsimd.collective_compute(
                    "AllReduce", OP.add,
                    replica_groups=[list(range(NCORE))],
                    ins=[cc_in[:]], outs=[cc_out[:]])
                sall = smp.tile([8, 2 * C], f32, tag="sall")
                nc.sync.dma_start(sall[:], cc_out[:])
                gw = gn_sb[:, 0:C]
                gb = gn_sb[:, C:2 * C]
                gms = gn_sb[:, 2 * C:3 * C]
                gms2m = gn_sb[:, 3 * C:4 * C]
                invc = gn_sb[:, 4 * C:4 * C + 1]
                epsc = gn_sb[:, 4 * C + 1:4 * C + 2]
                mean = smp.tile([8, C], f32, tag="mean")
                nc.vector.tensor_scalar(out=mean[:], in0=sall[:, 0:C],
                                        scalar1=invc, scalar2=None, op0=OP.mult)
                eh2 = smp.tile([8, C], f32, tag="eh2")
                nc.vector.tensor_scalar(out=eh2[:], in0=sall[:, C:2 * C],
                                        scalar1=invc, scalar2=None, op0=OP.mult)
                msq = smp.tile([8, C], f32, tag="msq")
                nc.vector.tensor_tensor(out=msq[:], in0=mean[:], in1=mean[:],
                                        op=OP.mult)
                var = smp.tile([8, C], f32, tag="var")
                nc.vector.tensor_tensor(out=msq[:], in0=msq[:], in1=gms2m,
                                        op=OP.mult)
                nc.vector.tensor_tensor(out=var[:], in0=eh2[:], in1=msq[:],
                                        op=OP.subtract)
                std = smp.tile([8, C], f32, tag="std")
                nc.scalar.activation(out=std[:], in_=var[:], func=AF.Sqrt,
                                     bias=epsc)
                ab = smp.tile([8, 2 * C], f32, tag="ab")
                nc.vector.reciprocal(std[:], std[:])
                nc.vector.tensor_tensor(out=ab[:, 0:C], in0=gw, in1=std[:],
                                        op=OP.mult)
                tm = smp.tile([8, C], f32, tag="tm")
                nc.vector.tensor_tensor(out=tm[:], in0=ab[:, 0:C], in1=mean[:],
                                        op=OP.mult)
                nc.vector.tensor_tensor(out=tm[:], in0=tm[:], in1=gms,
                                        op=OP.mult)
                nc.vector.tensor_tensor(out=ab[:, C:2 * C], in0=gb, in1=tm[:],
                                        op=OP.subtract)

                for g0 in range(0, TILES, G):
                    ng = min(G, TILES - g0)
                    obuf = ohp.tile([128, G * C], f32, tag="ob")
                    oht = ohp.tile([8, G * 128], f32, tag="oht")
                    nc.sync.dma_start(oht[:, 0:ng * 128],
                                      onehotT[:, g0 * 128:(g0 + ng) * 128])
                    for i in range(ng):
                        t = g0 + i
                        abpe = ps3.tile([128, 2 * C], f32, tag="abpe")
                        nc.tensor.matmul(abpe[:],
                                         lhsT=oht[:, i * 128:(i + 1) * 128],
                                         rhs=ab[:], start=True, stop=True)
                        nrm = smp.tile([128, C], f32, tag="nrm")
                        nc.vector.tensor_tensor(out=nrm[:],
                                                in0=h_sb[:, t * C:(t + 1) * C],
                                                in1=abpe[:, 0:C], op=OP.mult)
                        nc.vector.tensor_tensor(out=nrm[:], in0=nrm[:],
                                                in1=abpe[:, C:2 * C], op=OP.add)
                        nc.scalar.activation(out=obuf[:, i * C:(i + 1) * C],
                                             in_=nrm[:],
                                             func=AF.Gelu_apprx_tanh)
                    nfull = ng if g0 + ng < TILES else ng - 1
                    if nfull > 0:
                        nc.sync.dma_start(
                            out[g0 * 128:(g0 + nfull) * 128, :]
                            .rearrange("(g p) c -> p g c", p=128),
                            obuf[:, 0:nfull * C]
                            .rearrange("p (g c) -> p g c", c=C))
                    if g0 + ng == TILES:
                        nc.sync.dma_start(
                            out[(TILES - 1) * 128:(TILES - 1) * 128 + LAST_ROWS, :],
                            obuf[0:LAST_ROWS, (ng - 1) * C:ng * C])

    nc.compile()
    return nc


def kernel(**inputs):
    from concourse.bass_utils import run_bass_kernel_spmd

    cfg, in_maps = _host_prep(**inputs)
    nc = _build_nc(cfg)
    res = run_bass_kernel_spmd(nc, in_maps, core_ids=list(range(NCORE)))
    full = np.empty((N, C), _F32)
    for k in range(NCORE):
        full[cfg["own_all"][k]] = res.results[k]["out"]
    return full
